# revision 11
# baseline (speedup 1.0000x reference)
"""Multi-head attention (B=2, S=4096, D=768, H=12) on 8 Trainium2 cores.

Sharding: core c handles batch c//4 and heads 3*(c%4)..3*(c%4)+2.
Each core computes its 3 heads end-to-end; the host sums the 4 per-batch
partials and adds the output bias.

Fully software-pipelined single pass per core:
  iteration i emits projection of sequence block i (Q/K/V, PE-dense)
  interleaved with attention of query group i-1 (ScalarE-dense exps),
  the softmax normalize of group i-2, and the output projection of
  group i-3, so TensorE work hides under the exp-bound stretches.

  - Q^T/K^T [e,s]: heads 0/1 packed on partitions 0-63/64-127 of slot 0;
    head 2 on both halves of slot 1 (one half via SBUF->SBUF DMA dup) so
    its score matmuls pair across PE row groups like heads 0/1 do.
  - ec1 projection of Q-head2/K-head2 merged into one [128,512] psum fill.
  - scores: [128k, 2, 512q] fp32 PSUM tiles (2 banks); one exp (N=1024)
    per (head, key-block-pair) -> bf16; diagonal blocks masked 0/1 on DVE.
  - ctx^T + softmax denominator in one accumulating matmul per block:
    lhsT=[V|1] (65 cols) -> psum rows 0-63 ctx^T, row 64 = sum(exp).
  - denominator reciprocal broadcast via K=1 matmul into a score-pool
    bank, then one DVE multiply normalizes into bf16 ctx^T.
  - output projection from ctx^T against wo^T slices, PSUM tiles sharing
    the ctx slots.
PSUM: 2x2 score banks + 3 ctx/outproj banks + 1 projection bank = 8.
"""

import sys

sys.path.insert(0, "/opt/trn_rl_repo")

import ml_dtypes
import numpy as np

import concourse.bass as bass
import concourse.mybir as mybir
import concourse.tile as tile
from concourse.bass_utils import run_bass_kernel_spmd

B, S, D, H = 2, 4096, 768, 12
DK = D // H          # 64
NCORES = 8
HPC = 3              # heads per core
E = HPC * DK         # 192 = per-core projection width
P = 128
DC = D // P          # 6 contraction chunks of 128
SG = S // 512        # 8 query groups of 512
SC = S // P          # 32 token chunks of 128
F32 = mybir.dt.float32
F32R = mybir.dt.float32r
BF16 = mybir.dt.bfloat16
EXP = mybir.ActivationFunctionType.Exp
BF = ml_dtypes.bfloat16


def _split_multi_waits(nc):
    """This walrus build encodes exactly one sync wait per TPB instruction
    and refuses to split multi-wait instructions itself. Rewrite each block
    so extra waits land on same-engine NOPs directly before the owner."""
    k = 0
    for f in nc.m.functions:
        for blk in f.blocks:
            out = []
            changed = False
            for inst in blk.instructions:
                si = inst.sync_info
                if si is not None and len(si.on_wait) > 1:
                    changed = True
                    waits = list(si.on_wait)
                    for w in waits[:-1]:
                        nop = mybir.InstNoOp(name=f"splitw-{k}", ins=[], outs=[])
                        k += 1
                        nop.engine = inst.engine
                        nop.sync_info = mybir.SyncInfo(on_wait=[w], on_update=[])
                        out.append(nop)
                    inst.sync_info = mybir.SyncInfo(
                        on_wait=[waits[-1]], on_update=list(si.on_update)
                    )
                out.append(inst)
            if changed:
                blk.instructions = out


def _r(ap):
    return ap.bitcast(F32R)


def _build_program(repeat=1, parts="all"):
    nc = bass.Bass("TRN2", target_bir_lowering=False, debug=False)

    qT = nc.declare_dram_parameter("qT", [D, S], BF16, isOutput=False)
    kT = nc.declare_dram_parameter("kT", [D, S], BF16, isOutput=False)
    vT = nc.declare_dram_parameter("vT", [D, S], BF16, isOutput=False)
    wq0 = nc.declare_dram_parameter("wq0", [D, P], BF16, isOutput=False)
    wk0 = nc.declare_dram_parameter("wk0", [D, P], BF16, isOutput=False)
    wqk1 = nc.declare_dram_parameter("wqk1", [D, P], BF16, isOutput=False)
    wvT = nc.declare_dram_parameter("wvT", [D, 192], BF16, isOutput=False)
    woT = nc.declare_dram_parameter("woT", [E, D], BF16, isOutput=False)
    # per-partition bias columns (Q side pre-scaled by 1/sqrt(dk))
    bq0 = nc.declare_dram_parameter("bq0", [P, 1], F32, isOutput=False)
    bk0 = nc.declare_dram_parameter("bk0", [P, 1], F32, isOutput=False)
    bqk1 = nc.declare_dram_parameter("bqk1", [P, 1], F32, isOutput=False)
    bv = nc.declare_dram_parameter("bv", [P, 192], F32, isOutput=False)
    maskc = nc.declare_dram_parameter("maskc", [P, 4 * 512], BF16, isOutput=False)
    out_p = nc.declare_dram_parameter("out_p", [S, D], BF16, isOutput=True)

    qT_r = qT[:].rearrange("(dc p) s -> p dc s", p=P)
    kT_r = kT[:].rearrange("(dc p) s -> p dc s", p=P)
    vT_r = vT[:].rearrange("(dc p) s -> p dc s", p=P)

    with tile.TileContext(nc) as tc:
        import contextlib

        with contextlib.ExitStack() as ctx:
            const = ctx.enter_context(tc.tile_pool(name="const", bufs=1))
            persist = ctx.enter_context(tc.tile_pool(name="persist", bufs=1))

            # ---- constants ----
            wq0_sb = const.tile([P, DC, P], BF16)
            nc.sync.dma_start(wq0_sb[:], wq0[:].rearrange("(dc p) e -> p dc e", p=P))
            wk0_sb = const.tile([P, DC, P], BF16)
            nc.sync.dma_start(wk0_sb[:], wk0[:].rearrange("(dc p) e -> p dc e", p=P))
            wqk1_sb = const.tile([P, DC, P], BF16)
            nc.sync.dma_start(
                wqk1_sb[:], wqk1[:].rearrange("(dc p) e -> p dc e", p=P)
            )
            wv_sb = const.tile([P, DC, 192], BF16)
            nc.sync.dma_start(wv_sb[:], wvT[:].rearrange("(dc p) e -> p dc e", p=P))
            wo_sb = const.tile([64, HPC, D], BF16)
            nc.sync.dma_start(wo_sb[:], woT[:].rearrange("(h p) o -> p h o", p=64))
            bq0_sb = const.tile([P, 1], F32)
            nc.sync.dma_start(bq0_sb[:], bq0[:])
            bk0_sb = const.tile([P, 1], F32)
            nc.sync.dma_start(bk0_sb[:], bk0[:])
            bqk1_sb = const.tile([P, 1], F32)
            nc.sync.dma_start(bqk1_sb[:], bqk1[:])
            bv_sb = const.tile([P, 192], F32)
            nc.sync.dma_start(bv_sb[:], bv[:])
            mask_sb = const.tile([P, 4 * 512], BF16)
            nc.sync.dma_start(mask_sb[:], maskc[:])
            onesf = const.tile([P, 64], F32)
            nc.any.memset(onesf[:], 1.0)
            ones_sb = const.tile([P, 64], F32R)
            nc.vector.tensor_copy(ones_sb[:], onesf[:])

            # ---- persistent activations ----
            # slot 0: heads 0 (p0-63) & 1 (p64-127); slot 1: head 2 on BOTH
            # halves (dup) so head-2 score MMs pair across PE row groups.
            QT_sb = persist.tile([P, 2, S], BF16)
            KT_sb = persist.tile([P, 2, S], BF16)
            V_all = persist.tile([P, SC, HPC, 65], BF16)
            ctxT_sb = persist.tile([64, HPC, S], BF16)
            nc.any.memset(V_all[:, :, :, 64], 1.0)

            head_loc = [(0, 0), (0, 64), (1, 0)]  # (slot, base part) h0/h1

            with tc.tile_pool(name="pj", bufs=2) as pj, \
                 tc.tile_pool(name="att", bufs=4) as att, \
                 tc.tile_pool(name="nrm", bufs=3) as nrm, \
                 tc.tile_pool(name="ob", bufs=3) as ob, \
                 tc.tile_pool(name="pjp", bufs=1, space="PSUM") as pjp, \
                 tc.tile_pool(name="stp", bufs=2, space="PSUM") as stp, \
                 tc.tile_pool(name="ctxp", bufs=3, space="PSUM") as ctxp:

                def proj_steps(sg):
                    """Generator: one PSUM fill (or the staging DMAs) per
                    next(); all writes for sequence block sg."""
                    win = slice(sg * 512, (sg + 1) * 512)
                    qstg = pj.tile([P, DC, 512], BF16, tag="qstg", name="qstg")
                    nc.sync.dma_start(qstg[:], qT_r[:, :, win])
                    kstg = pj.tile([P, DC, 512], BF16, tag="kstg", name="kstg")
                    nc.sync.dma_start(kstg[:], kT_r[:, :, win])
                    vstg = pj.tile([P, DC, 512], BF16, tag="vstg", name="vstg")
                    nc.sync.dma_start(vstg[:], vT_r[:, :, win])
                    yield
                    # Q ec0: heads 0/1 -> QT slot 0 (all 128 partitions)
                    ps = pjp.tile([P, 512], F32, tag="pj", name="psq")
                    for dc in range(DC):
                        nc.tensor.matmul(
                            ps[:], wq0_sb[:, dc, :], qstg[:, dc, :],
                            start=(dc == 0), stop=(dc == DC - 1),
                        )
                    nc.vector.tensor_scalar_add(
                        QT_sb[:, 0, win], ps[:], bq0_sb[:, 0:1]
                    )
                    yield
                    # K ec0
                    ps = pjp.tile([P, 512], F32, tag="pj", name="psk")
                    for dc in range(DC):
                        nc.tensor.matmul(
                            ps[:], wk0_sb[:, dc, :], kstg[:, dc, :],
                            start=(dc == 0), stop=(dc == DC - 1),
                        )
                    nc.vector.tensor_scalar_add(
                        KT_sb[:, 0, win], ps[:], bk0_sb[:, 0:1]
                    )
                    yield
                    # ec1 (head 2): two 6-MM chains into different column
                    # groups of ONE psum bank, executing concurrently on PE.
                    # Rows 0-63 = Q head2 (from qstg), rows 64-127 = K head2
                    # (from kstg) -- K lands directly in its dup position.
                    ps = pjp.tile([P, 512], F32, tag="pj", name="psqk")
                    for dc in range(DC):
                        nc.tensor.matmul(
                            ps[0:64, :], wqk1_sb[:, dc, 0:64],
                            qstg[:, dc, :],
                            start=(dc == 0), stop=(dc == DC - 1),
                        )
                        nc.tensor.matmul(
                            ps[64:128, :], wqk1_sb[:, dc, 64:128],
                            kstg[:, dc, :],
                            start=(dc == 0), stop=(dc == DC - 1),
                        )
                    nc.vector.tensor_scalar_add(
                        QT_sb[0:64, 1, win], ps[0:64, :], bqk1_sb[0:64, 0:1]
                    )
                    nc.vector.tensor_scalar_add(
                        KT_sb[64:128, 1, win], ps[64:128, :],
                        bqk1_sb[64:128, 0:1]
                    )
                    # duplicate across halves for paired head-2 score MMs
                    nc.sync.dma_start(QT_sb[64:128, 1, win], QT_sb[0:64, 1, win])
                    nc.sync.dma_start(KT_sb[0:64, 1, win], KT_sb[64:128, 1, win])
                    yield
                    # V: two [128,384] fills, each covering 2 token chunks
                    for u2 in range(2):
                        ps = pjp.tile([P, 512], F32, tag="pj", name="psv")
                        for u in range(2):
                            for dc in range(DC):
                                nc.tensor.matmul(
                                    ps[:, u * 192:(u + 1) * 192],
                                    vstg[:, dc,
                                         (2 * u2 + u) * P:(2 * u2 + u + 1) * P],
                                    wv_sb[:, dc, :],
                                    start=(dc == 0), stop=(dc == DC - 1),
                                )
                        for u in range(2):
                            t = 4 * sg + 2 * u2 + u
                            nc.vector.tensor_tensor(
                                V_all[:, t, :, 0:64],
                                ps[:, u * 192:(u + 1) * 192].rearrange(
                                    "p (h e) -> p h e", h=HPC
                                ),
                                bv_sb[:].rearrange(
                                    "p (h e) -> p h e", h=HPC
                                ),
                                mybir.AluOpType.add,
                            )
                        if u2 == 0:
                            yield

                def emit_scores_exps(qg, pr):
                    qwin = QT_sb[:, :, qg * 512:(qg + 1) * 512]
                    kbs = (2 * pr, 2 * pr + 1)
                    sts = {}
                    for h in (0, 1):
                        sts[h] = stp.tile([P, 2, 512], F32, tag="st",
                                          name=f"st{h}")
                    for j, kb in enumerate(kbs):
                        for h in (0, 1):
                            slot, p0 = head_loc[h]
                            nc.tensor.matmul(
                                sts[h][:, j, :],
                                KT_sb[p0:p0 + 64, slot, kb * P:(kb + 1) * P],
                                qwin[p0:p0 + 64, slot, :],
                                start=True, stop=True,
                            )
                    ets = {}
                    for h in (0, 1):
                        ets[h] = att.tile([P, 2, 512], BF16, tag="et",
                                          name=f"et{h}")
                        nc.scalar.activation(ets[h][:], sts[h][:], EXP)
                    sts[2] = stp.tile([P, 2, 512], F32, tag="st", name="st2")
                    for j, kb in enumerate(kbs):
                        p0 = 64 * j
                        nc.tensor.matmul(
                            sts[2][:, j, :],
                            KT_sb[p0:p0 + 64, 1, kb * P:(kb + 1) * P],
                            qwin[p0:p0 + 64, 1, :],
                            start=True, stop=True,
                        )
                    ets[2] = att.tile([P, 2, 512], BF16, tag="et", name="et2")
                    nc.scalar.activation(ets[2][:], sts[2][:], EXP)
                    return ets

                def emit_mask_ctx(qg, pr, ets, ctx_ps):
                    nkb = 4 * (qg + 1)
                    kbs = (2 * pr, 2 * pr + 1)
                    for j, kb in enumerate(kbs):
                        if kb >= nkb - 4:
                            dj = kb - (nkb - 4)
                            for h in range(HPC):
                                em = att.tile([P, 512], BF16, tag="etm",
                                              name="em")
                                nc.vector.tensor_tensor(
                                    em[:], ets[h][:, j, :],
                                    mask_sb[:, dj * 512:(dj + 1) * 512],
                                    mybir.AluOpType.mult,
                                )
                                ets[(h, j)] = em
                    for h in range(HPC):
                        for j, kb in enumerate(kbs):
                            src = ets.get((h, j), None)
                            esl = (src[:] if src is not None
                                   else ets[h][:, j, :])
                            nc.tensor.matmul(
                                ctx_ps[h][0:65, :],
                                V_all[:, kb, h, :],
                                esl,
                                start=(kb == 0), stop=(kb == nkb - 1),
                            )

                def emit_normalize(qg, ctx_ps):
                    # reciprocal of the denominator row (~51 ULP is plenty),
                    # K=1 matmul broadcast across partitions (bank borrowed
                    # from the score pool), then one DVE multiply normalizes
                    # into bf16 ctx^T.
                    for h in range(HPC):
                        rc = nrm.tile([P, 512], F32R, tag="rc", name="rc")
                        with nc.allow_low_precision(
                            reason="softmax denominator reciprocal; f32r "
                            "rounding is benign here"
                        ):
                            nc.vector.reciprocal(
                                rc[64:65, :], ctx_ps[h][64:65, :]
                            )
                        bc = stp.tile([P, 2, 512], F32, tag="st", name="bc")
                        nc.tensor.matmul(
                            bc[0:64, 0, :], _r(ones_sb[64:65, :]),
                            rc[64:65, :], start=True, stop=True,
                        )
                        rcb = nrm.tile([64, 512], F32, tag="rcb", name="rcb")
                        nc.vector.tensor_copy(rcb[:], bc[0:64, 0, :])
                        nc.vector.tensor_tensor(
                            ctxT_sb[0:64, h, qg * 512:(qg + 1) * 512],
                            ctx_ps[h][0:64, :],
                            rcb[:],
                            mybir.AluOpType.mult,
                        )

                def emit_outproj(src_qg):
                    for sc in range(4 * src_qg, 4 * src_qg + 4):
                        osb = ob.tile([P, D], BF16, tag="osb", name="osb")
                        for og, o0, ow in ((0, 0, 512), (1, 512, 256)):
                            ps = ctxp.tile([P, 512], F32, tag="ctx",
                                           name=f"og{og}")
                            for h in range(HPC):
                                nc.tensor.matmul(
                                    ps[:, :ow],
                                    ctxT_sb[0:64, h, sc * P:(sc + 1) * P],
                                    wo_sb[:, h, o0:o0 + ow],
                                    start=(h == 0), stop=(h == HPC - 1),
                                )
                            nc.vector.tensor_copy(
                                osb[:, o0:o0 + ow], ps[:, :ow]
                            )
                        nc.sync.dma_start(
                            out_p[sc * P:(sc + 1) * P, :], osb[:]
                        )

                for _rep in range(repeat):
                    # iteration 0: projection of block 0, standalone
                    for _ in proj_steps(0):
                        pass
                    prev_ctx = None
                    for qg in range(SG):
                        nkb = 4 * (qg + 1)
                        steps = proj_steps(qg + 1) if qg + 1 < SG else iter(())
                        ets0 = emit_scores_exps(qg, 0)
                        if prev_ctx is not None:
                            emit_normalize(qg - 1, prev_ctx)
                        if qg > 1:
                            emit_outproj(qg - 2)
                        ctx_ps = {}
                        for h in range(HPC):
                            ctx_ps[h] = ctxp.tile(
                                [P, 512], F32, tag="ctx", name=f"ctx{h}"
                            )
                        emit_mask_ctx(qg, 0, ets0, ctx_ps)
                        for pr in range(1, nkb // 2):
                            next(steps, None)
                            ets = emit_scores_exps(qg, pr)
                            emit_mask_ctx(qg, pr, ets, ctx_ps)
                            if qg == SG - 1 and pr == nkb // 4:
                                # drain the second-to-last group's output
                                # projection mid-loop to shorten the tail
                                emit_outproj(SG - 2)
                        for _ in steps:
                            pass
                        prev_ctx = ctx_ps
                    emit_normalize(SG - 1, prev_ctx)
                    emit_outproj(SG - 1)

    _split_multi_waits(nc)
    return nc


_CACHED_NC = None


def _get_nc():
    global _CACHED_NC
    if _CACHED_NC is None:
        _CACHED_NC = _build_program()
    return _CACHED_NC


def _numpy_reference(q, k, v, wq, bq, wk, bk, wv, bv, wo, bo, mask):
    """Fallback for masks the fast path does not handle (non-causal)."""
    out = np.empty((B, S, D), np.float32)
    scale = 1.0 / np.sqrt(DK)
    for b in range(B):
        Q = (q[b] @ wq.T + bq).reshape(S, H, DK).transpose(1, 0, 2)
        K = (k[b] @ wk.T + bk).reshape(S, H, DK).transpose(1, 0, 2)
        V = (v[b] @ wv.T + bv).reshape(S, H, DK).transpose(1, 0, 2)
        ctx = np.empty((H, S, DK), np.float32)
        for h in range(H):
            s = (Q[h] @ K[h].T) * scale
            s = np.where(mask, s, -1e9)
            s -= s.max(axis=-1, keepdims=True)
            e = np.exp(s)
            p = e / e.sum(axis=-1, keepdims=True)
            ctx[h] = p @ V[h]
        out[b] = ctx.transpose(1, 0, 2).reshape(S, D) @ wo.T + bo
    return out


def _prepare_in_maps(q, k, v, wq, bq, wk, bk, wv, bv, wo):
    # causal 0/1 diagonal-block masks: maskc[k, j*512+q] = (128j + k) <= q
    kk = np.arange(P)[:, None]
    qq = np.arange(512)[None, :]
    maskc = np.zeros((P, 4, 512), np.float32)
    for j in range(4):
        maskc[:, j, :] = (P * j + kk) <= qq
    maskc = np.ascontiguousarray(maskc.reshape(P, 4 * 512)).astype(BF)

    scale = 1.0 / np.sqrt(DK)
    wqT = np.ascontiguousarray(wq.T) * scale      # [d_in, e_out], pre-scaled
    wkT = np.ascontiguousarray(wk.T)
    wvT = np.ascontiguousarray(wv.T)
    woT = np.ascontiguousarray(wo.T)              # [e_in, d_out]

    qTb = [np.ascontiguousarray(q[b].T).astype(BF) for b in range(B)]
    kTb = [np.ascontiguousarray(k[b].T).astype(BF) for b in range(B)]
    vTb = [np.ascontiguousarray(v[b].T).astype(BF) for b in range(B)]

    in_maps = []
    for c in range(NCORES):
        b = c // 4
        e0 = 3 * (c % 4) * DK
        wq_c = wqT[:, e0:e0 + E]
        wk_c = wkT[:, e0:e0 + E]
        bq_c = bq[e0:e0 + E] * scale
        bk_c = bk[e0:e0 + E]
        wqk1 = np.concatenate([wq_c[:, P:E], wk_c[:, P:E]], axis=1)
        bqk1 = np.concatenate([bq_c[P:E], bk_c[P:E]])[:, None]
        wvp = np.ascontiguousarray(wvT[:, e0:e0 + E]).astype(BF)
        bvp = np.zeros((P, 192), np.float32)
        bvp[:E, :] = 0.0
        bvp = np.ascontiguousarray(
            np.broadcast_to(bv[e0:e0 + E][None, :], (P, 192))
        ).astype(np.float32)
        in_maps.append({
            "qT": qTb[b],
            "kT": kTb[b],
            "vT": vTb[b],
            "wq0": np.ascontiguousarray(wq_c[:, 0:P]).astype(BF),
            "wk0": np.ascontiguousarray(wk_c[:, 0:P]).astype(BF),
            "wqk1": np.ascontiguousarray(wqk1).astype(BF),
            "wvT": wvp,
            "woT": np.ascontiguousarray(woT[e0:e0 + E, :]).astype(BF),
            "bq0": np.ascontiguousarray(bq_c[0:P])[:, None].astype(np.float32),
            "bk0": np.ascontiguousarray(bk_c[0:P])[:, None].astype(np.float32),
            "bqk1": np.ascontiguousarray(bqk1).astype(np.float32),
            "bv": bvp,
            "maskc": maskc,
        })
    return in_maps


def kernel(q, k, v, wq, bq, wk, bk, wv, bv, wo, bo, mask, **_unused):
    q = np.asarray(q, np.float32)
    k = np.asarray(k, np.float32)
    v = np.asarray(v, np.float32)
    wq = np.asarray(wq, np.float32)
    wk = np.asarray(wk, np.float32)
    wv = np.asarray(wv, np.float32)
    wo = np.asarray(wo, np.float32)
    bq = np.asarray(bq, np.float32)
    bk = np.asarray(bk, np.float32)
    bv = np.asarray(bv, np.float32)
    bo = np.asarray(bo, np.float32)
    mask = np.asarray(mask)

    tril = np.tril(np.ones((S, S), bool))
    if mask.shape != (S, S) or not np.array_equal(mask.astype(bool), tril):
        return _numpy_reference(q, k, v, wq, bq, wk, bk, wv, bv, wo, bo, mask)

    in_maps = _prepare_in_maps(q, k, v, wq, bq, wk, bk, wv, bv, wo)
    nc = _get_nc()
    res = run_bass_kernel_spmd(nc, in_maps, core_ids=list(range(NCORES)))

    out = np.empty((B, S, D), np.float32)
    for b in range(B):
        acc = res.results[4 * b]["out_p"].astype(np.float32)
        for c in range(4 * b + 1, 4 * b + 4):
            acc = acc + res.results[c]["out_p"].astype(np.float32)
        out[b] = acc + bo[None, :]
    return out


# revision 12
# speedup vs baseline: 1.4262x; 1.4262x over previous
"""Multi-head attention (B=2, S=4096, D=768, H=12) on 8 Trainium2 cores.

Sharding: core c handles batch c//4 and heads 3*(c%4)..3*(c%4)+2.
Each core computes its 3 heads end-to-end; the host sums the 4 per-batch
partials and adds the output bias.

Fully software-pipelined single pass per core:
  iteration i emits projection of sequence block i (Q/K/V, PE-dense)
  interleaved with attention of query group i-1 (ScalarE-dense exps),
  the softmax normalize of group i-2, and the output projection of
  group i-3, so TensorE work hides under the exp-bound stretches.

  - Q^T/K^T [e,s]: heads 0/1 packed on partitions 0-63/64-127 of slot 0;
    head 2 on both halves of slot 1 (one half via SBUF->SBUF DMA dup) so
    its score matmuls pair across PE row groups like heads 0/1 do.
  - ec1 projection of Q-head2/K-head2 merged into one [128,512] psum fill.
  - scores: [128k, 2, 512q] fp32 PSUM tiles (2 banks); one exp (N=1024)
    per (head, key-block-pair) -> bf16; diagonal blocks masked 0/1 on DVE.
  - ctx^T + softmax denominator in one accumulating matmul per block:
    lhsT=[V|1] (65 cols) -> psum rows 0-63 ctx^T, row 64 = sum(exp).
  - denominator reciprocal broadcast via K=1 matmul into a score-pool
    bank, then one DVE multiply normalizes into bf16 ctx^T.
  - output projection from ctx^T against wo^T slices, PSUM tiles sharing
    the ctx slots.
PSUM: 2x2 score banks + 3 ctx/outproj banks + 1 projection bank = 8.
"""

import sys

sys.path.insert(0, "/opt/trn_rl_repo")

import ml_dtypes
import numpy as np

import concourse.bass as bass
import concourse.mybir as mybir
import concourse.tile as tile
from concourse.bass_utils import run_bass_kernel_spmd

B, S, D, H = 2, 4096, 768, 12
DK = D // H          # 64
NCORES = 8
HPC = 3              # heads per core
E = HPC * DK         # 192 = per-core projection width
P = 128
DC = D // P          # 6 contraction chunks of 128
SG = S // 512        # 8 query groups of 512
SC = S // P          # 32 token chunks of 128
F32 = mybir.dt.float32
F32R = mybir.dt.float32r
BF16 = mybir.dt.bfloat16
EXP = mybir.ActivationFunctionType.Exp
BF = ml_dtypes.bfloat16


def _split_multi_waits(nc):
    """This walrus build encodes exactly one sync wait per TPB instruction
    and refuses to split multi-wait instructions itself. Rewrite each block
    so extra waits land on same-engine NOPs directly before the owner."""
    k = 0
    for f in nc.m.functions:
        for blk in f.blocks:
            out = []
            changed = False
            for inst in blk.instructions:
                si = inst.sync_info
                if si is not None and len(si.on_wait) > 1:
                    changed = True
                    waits = list(si.on_wait)
                    for w in waits[:-1]:
                        nop = mybir.InstNoOp(name=f"splitw-{k}", ins=[], outs=[])
                        k += 1
                        nop.engine = inst.engine
                        nop.sync_info = mybir.SyncInfo(on_wait=[w], on_update=[])
                        out.append(nop)
                    inst.sync_info = mybir.SyncInfo(
                        on_wait=[waits[-1]], on_update=list(si.on_update)
                    )
                out.append(inst)
            if changed:
                blk.instructions = out


def _r(ap):
    return ap.bitcast(F32R)


def _build_program(repeat=1, parts="all"):
    nc = bass.Bass("TRN2", target_bir_lowering=False, debug=False)

    qT = nc.declare_dram_parameter("qT", [D, S], BF16, isOutput=False)
    kT = nc.declare_dram_parameter("kT", [D, S], BF16, isOutput=False)
    vT = nc.declare_dram_parameter("vT", [D, S], BF16, isOutput=False)
    wq0 = nc.declare_dram_parameter("wq0", [D, P], BF16, isOutput=False)
    wk0 = nc.declare_dram_parameter("wk0", [D, P], BF16, isOutput=False)
    wqk1 = nc.declare_dram_parameter("wqk1", [D, P], BF16, isOutput=False)
    wvT = nc.declare_dram_parameter("wvT", [D, 192], BF16, isOutput=False)
    woT = nc.declare_dram_parameter("woT", [E, D], BF16, isOutput=False)
    # per-partition bias columns (Q side pre-scaled by 1/sqrt(dk))
    bq0 = nc.declare_dram_parameter("bq0", [P, 1], F32, isOutput=False)
    bk0 = nc.declare_dram_parameter("bk0", [P, 1], F32, isOutput=False)
    bqk1 = nc.declare_dram_parameter("bqk1", [P, 1], F32, isOutput=False)
    bv = nc.declare_dram_parameter("bv", [P, 192], F32, isOutput=False)
    maskc = nc.declare_dram_parameter("maskc", [P, 4 * 512], BF16, isOutput=False)
    out_p = nc.declare_dram_parameter("out_p", [S, D], BF16, isOutput=True)

    qT_r = qT[:].rearrange("(dc p) s -> p dc s", p=P)
    kT_r = kT[:].rearrange("(dc p) s -> p dc s", p=P)
    vT_r = vT[:].rearrange("(dc p) s -> p dc s", p=P)

    with tile.TileContext(nc) as tc:
        import contextlib

        with contextlib.ExitStack() as ctx:
            const = ctx.enter_context(tc.tile_pool(name="const", bufs=1))
            persist = ctx.enter_context(tc.tile_pool(name="persist", bufs=1))

            # ---- constants ----
            wq0_sb = const.tile([P, DC, P], BF16)
            nc.sync.dma_start(wq0_sb[:], wq0[:].rearrange("(dc p) e -> p dc e", p=P))
            wk0_sb = const.tile([P, DC, P], BF16)
            nc.sync.dma_start(wk0_sb[:], wk0[:].rearrange("(dc p) e -> p dc e", p=P))
            wqk1_sb = const.tile([P, DC, P], BF16)
            nc.sync.dma_start(
                wqk1_sb[:], wqk1[:].rearrange("(dc p) e -> p dc e", p=P)
            )
            bq0_sb = const.tile([P, 1], F32)
            nc.sync.dma_start(bq0_sb[:], bq0[:])
            bk0_sb = const.tile([P, 1], F32)
            nc.sync.dma_start(bk0_sb[:], bk0[:])
            bqk1_sb = const.tile([P, 1], F32)
            nc.sync.dma_start(bqk1_sb[:], bqk1[:])

            # constants not needed by the first Q/K projection are emitted
            # AFTER the first staging DMAs (the HWDGE ring is FIFO, so this
            # keeps ~2.5 MB of weight traffic off the startup critical path)
            wv_sb = wo_sb = bv_sb = mask_sb = ones_sb = None

            def emit_late_consts():
                nonlocal wv_sb, wo_sb, bv_sb, mask_sb, ones_sb
                wv_sb = const.tile([P, DC, 192], BF16, name="wv_sb")
                nc.sync.dma_start(
                    wv_sb[:], wvT[:].rearrange("(dc p) e -> p dc e", p=P)
                )
                mask_sb = const.tile([P, 4 * 512], BF16, name="mask_sb")
                nc.sync.dma_start(mask_sb[:], maskc[:])
                bv_sb = const.tile([P, 192], F32, name="bv_sb")
                nc.sync.dma_start(bv_sb[:], bv[:])
                wo_sb = const.tile([64, HPC, D], BF16, name="wo_sb")
                nc.sync.dma_start(
                    wo_sb[:], woT[:].rearrange("(h p) o -> p h o", p=64)
                )
                onesf = const.tile([P, 64], F32, name="onesf")
                nc.any.memset(onesf[:], 1.0)
                ones_sb = const.tile([P, 64], F32R, name="ones_sb")
                nc.vector.tensor_copy(ones_sb[:], onesf[:])

            # ---- persistent activations ----
            # slot 0: heads 0 (p0-63) & 1 (p64-127); slot 1: head 2 on BOTH
            # halves (dup) so head-2 score MMs pair across PE row groups.
            QT_sb = persist.tile([P, 2, S], BF16)
            KT_sb = persist.tile([P, 2, S], BF16)
            V_all = persist.tile([P, SC, HPC, 65], BF16)
            ctxT_sb = persist.tile([64, HPC, S], BF16)
            nc.any.memset(V_all[:, :, :, 64], 1.0)

            head_loc = [(0, 0), (0, 64), (1, 0)]  # (slot, base part) h0/h1

            with tc.tile_pool(name="pj", bufs=2) as pj, \
                 tc.tile_pool(name="att", bufs=4) as att, \
                 tc.tile_pool(name="nrm", bufs=3) as nrm, \
                 tc.tile_pool(name="ob", bufs=3) as ob, \
                 tc.tile_pool(name="pjp", bufs=1, space="PSUM") as pjp, \
                 tc.tile_pool(name="stp", bufs=2, space="PSUM") as stp, \
                 tc.tile_pool(name="ctxp", bufs=3, space="PSUM") as ctxp:

                def proj_steps(sg):
                    """Generator: one PSUM fill (or the staging DMAs) per
                    next(); all writes for sequence block sg."""
                    win = slice(sg * 512, (sg + 1) * 512)
                    qstg = pj.tile([P, DC, 512], BF16, tag="qstg", name="qstg")
                    nc.sync.dma_start(qstg[:], qT_r[:, :, win])
                    kstg = pj.tile([P, DC, 512], BF16, tag="kstg", name="kstg")
                    nc.sync.dma_start(kstg[:], kT_r[:, :, win])
                    vstg = pj.tile([P, DC, 512], BF16, tag="vstg", name="vstg")
                    nc.sync.dma_start(vstg[:], vT_r[:, :, win])
                    yield
                    # Q ec0: heads 0/1 -> QT slot 0 (all 128 partitions)
                    ps = pjp.tile([P, 512], F32, tag="pj", name="psq")
                    for dc in range(DC):
                        nc.tensor.matmul(
                            ps[:], wq0_sb[:, dc, :], qstg[:, dc, :],
                            start=(dc == 0), stop=(dc == DC - 1),
                        )
                    nc.vector.tensor_scalar_add(
                        QT_sb[:, 0, win], ps[:], bq0_sb[:, 0:1]
                    )
                    yield
                    # K ec0
                    ps = pjp.tile([P, 512], F32, tag="pj", name="psk")
                    for dc in range(DC):
                        nc.tensor.matmul(
                            ps[:], wk0_sb[:, dc, :], kstg[:, dc, :],
                            start=(dc == 0), stop=(dc == DC - 1),
                        )
                    nc.vector.tensor_scalar_add(
                        KT_sb[:, 0, win], ps[:], bk0_sb[:, 0:1]
                    )
                    yield
                    # ec1 (head 2): two 6-MM chains into different column
                    # groups of ONE psum bank, executing concurrently on PE.
                    # Rows 0-63 = Q head2 (from qstg), rows 64-127 = K head2
                    # (from kstg) -- K lands directly in its dup position.
                    ps = pjp.tile([P, 512], F32, tag="pj", name="psqk")
                    for dc in range(DC):
                        nc.tensor.matmul(
                            ps[0:64, :], wqk1_sb[:, dc, 0:64],
                            qstg[:, dc, :],
                            start=(dc == 0), stop=(dc == DC - 1),
                        )
                        nc.tensor.matmul(
                            ps[64:128, :], wqk1_sb[:, dc, 64:128],
                            kstg[:, dc, :],
                            start=(dc == 0), stop=(dc == DC - 1),
                        )
                    nc.vector.tensor_scalar_add(
                        QT_sb[0:64, 1, win], ps[0:64, :], bqk1_sb[0:64, 0:1]
                    )
                    nc.vector.tensor_scalar_add(
                        KT_sb[64:128, 1, win], ps[64:128, :],
                        bqk1_sb[64:128, 0:1]
                    )
                    # duplicate across halves for paired head-2 score MMs
                    nc.sync.dma_start(QT_sb[64:128, 1, win], QT_sb[0:64, 1, win])
                    nc.sync.dma_start(KT_sb[0:64, 1, win], KT_sb[64:128, 1, win])
                    yield
                    # V: two [128,384] fills, each covering 2 token chunks
                    for u2 in range(2):
                        ps = pjp.tile([P, 512], F32, tag="pj", name="psv")
                        for u in range(2):
                            for dc in range(DC):
                                nc.tensor.matmul(
                                    ps[:, u * 192:(u + 1) * 192],
                                    vstg[:, dc,
                                         (2 * u2 + u) * P:(2 * u2 + u + 1) * P],
                                    wv_sb[:, dc, :],
                                    start=(dc == 0), stop=(dc == DC - 1),
                                )
                        for u in range(2):
                            t = 4 * sg + 2 * u2 + u
                            nc.vector.tensor_tensor(
                                V_all[:, t, :, 0:64],
                                ps[:, u * 192:(u + 1) * 192].rearrange(
                                    "p (h e) -> p h e", h=HPC
                                ),
                                bv_sb[:].rearrange(
                                    "p (h e) -> p h e", h=HPC
                                ),
                                mybir.AluOpType.add,
                            )
                        if u2 == 0:
                            yield

                def emit_scores_exps(qg, pr):
                    qwin = QT_sb[:, :, qg * 512:(qg + 1) * 512]
                    kbs = (2 * pr, 2 * pr + 1)
                    sts = {}
                    for h in (0, 1):
                        sts[h] = stp.tile([P, 2, 512], F32, tag="st",
                                          name=f"st{h}")
                    for j, kb in enumerate(kbs):
                        for h in (0, 1):
                            slot, p0 = head_loc[h]
                            nc.tensor.matmul(
                                sts[h][:, j, :],
                                KT_sb[p0:p0 + 64, slot, kb * P:(kb + 1) * P],
                                qwin[p0:p0 + 64, slot, :],
                                start=True, stop=True,
                            )
                    ets = {}
                    for h in (0, 1):
                        ets[h] = att.tile([P, 2, 512], BF16, tag="et",
                                          name=f"et{h}")
                        nc.scalar.activation(ets[h][:], sts[h][:], EXP)
                    sts[2] = stp.tile([P, 2, 512], F32, tag="st", name="st2")
                    for j, kb in enumerate(kbs):
                        p0 = 64 * j
                        nc.tensor.matmul(
                            sts[2][:, j, :],
                            KT_sb[p0:p0 + 64, 1, kb * P:(kb + 1) * P],
                            qwin[p0:p0 + 64, 1, :],
                            start=True, stop=True,
                        )
                    ets[2] = att.tile([P, 2, 512], BF16, tag="et", name="et2")
                    nc.scalar.activation(ets[2][:], sts[2][:], EXP)
                    return ets

                def emit_mask_ctx(qg, pr, ets, ctx_ps):
                    nkb = 4 * (qg + 1)
                    kbs = (2 * pr, 2 * pr + 1)
                    # last pair's masks run on the idle Pool engine so the
                    # DVE queue reaches the normalize chain sooner at the
                    # group boundary (the score-tile rotation waits on it)
                    eng = nc.vector
                    for j, kb in enumerate(kbs):
                        if kb >= nkb - 4:
                            dj = kb - (nkb - 4)
                            for h in range(HPC):
                                em = att.tile([P, 512], BF16, tag="etm",
                                              name="em")
                                eng.tensor_tensor(
                                    em[:], ets[h][:, j, :],
                                    mask_sb[:, dj * 512:(dj + 1) * 512],
                                    mybir.AluOpType.mult,
                                )
                                ets[(h, j)] = em
                    for h in range(HPC):
                        for j, kb in enumerate(kbs):
                            src = ets.get((h, j), None)
                            esl = (src[:] if src is not None
                                   else ets[h][:, j, :])
                            nc.tensor.matmul(
                                ctx_ps[h][0:65, :],
                                V_all[:, kb, h, :],
                                esl,
                                start=(kb == 0), stop=(kb == nkb - 1),
                            )

                def emit_normalize(qg, ctx_ps):
                    # reciprocal of the denominator row (~51 ULP is plenty),
                    # K=1 matmul broadcast across partitions (bank borrowed
                    # from the score pool), then one DVE multiply normalizes
                    # into bf16 ctx^T.
                    for h in range(HPC):
                        rc = nrm.tile([P, 512], F32R, tag="rc", name="rc")
                        with nc.allow_low_precision(
                            reason="softmax denominator reciprocal; f32r "
                            "rounding is benign here"
                        ):
                            nc.vector.reciprocal(
                                rc[64:65, :], ctx_ps[h][64:65, :]
                            )
                        bc = stp.tile([P, 2, 512], F32, tag="st", name="bc")
                        nc.tensor.matmul(
                            bc[0:64, 0, :], _r(ones_sb[64:65, :]),
                            rc[64:65, :], start=True, stop=True,
                        )
                        rcb = nrm.tile([64, 512], F32, tag="rcb", name="rcb")
                        nc.vector.tensor_copy(rcb[:], bc[0:64, 0, :])
                        nc.vector.tensor_tensor(
                            ctxT_sb[0:64, h, qg * 512:(qg + 1) * 512],
                            ctx_ps[h][0:64, :],
                            rcb[:],
                            mybir.AluOpType.mult,
                        )

                def emit_outproj(src_qg):
                    for sc in range(4 * src_qg, 4 * src_qg + 4):
                        osb = ob.tile([P, D], BF16, tag="osb", name="osb")
                        for og, o0, ow in ((0, 0, 512), (1, 512, 256)):
                            ps = ctxp.tile([P, 512], F32, tag="ctx",
                                           name=f"og{og}")
                            for h in range(HPC):
                                nc.tensor.matmul(
                                    ps[:, :ow],
                                    ctxT_sb[0:64, h, sc * P:(sc + 1) * P],
                                    wo_sb[:, h, o0:o0 + ow],
                                    start=(h == 0), stop=(h == HPC - 1),
                                )
                            nc.vector.tensor_copy(
                                osb[:, o0:o0 + ow], ps[:, :ow]
                            )
                        nc.sync.dma_start(
                            out_p[sc * P:(sc + 1) * P, :], osb[:]
                        )

                for _rep in range(repeat):
                    # iteration 0: projection of block 0, standalone; its
                    # staging DMAs go out before the late constants
                    steps0 = proj_steps(0)
                    next(steps0)
                    if _rep == 0:
                        emit_late_consts()
                    for _ in steps0:
                        pass
                    prev_ctx = None
                    for qg in range(SG):
                        nkb = 4 * (qg + 1)
                        steps = proj_steps(qg + 1) if qg + 1 < SG else iter(())
                        ets0 = emit_scores_exps(qg, 0)
                        if prev_ctx is not None:
                            emit_normalize(qg - 1, prev_ctx)
                        if qg > 1:
                            emit_outproj(qg - 2)
                        ctx_ps = {}
                        for h in range(HPC):
                            ctx_ps[h] = ctxp.tile(
                                [P, 512], F32, tag="ctx", name=f"ctx{h}"
                            )
                        emit_mask_ctx(qg, 0, ets0, ctx_ps)
                        for pr in range(1, nkb // 2):
                            next(steps, None)
                            ets = emit_scores_exps(qg, pr)
                            emit_mask_ctx(qg, pr, ets, ctx_ps)
                            if qg == SG - 1 and pr == nkb // 4:
                                # drain the second-to-last group's output
                                # projection mid-loop to shorten the tail
                                emit_outproj(SG - 2)
                        for _ in steps:
                            pass
                        prev_ctx = ctx_ps
                    emit_normalize(SG - 1, prev_ctx)
                    emit_outproj(SG - 1)

    _split_multi_waits(nc)
    return nc


_CACHED_NC = None


def _get_nc():
    global _CACHED_NC
    if _CACHED_NC is None:
        _CACHED_NC = _build_program()
    return _CACHED_NC


def _numpy_reference(q, k, v, wq, bq, wk, bk, wv, bv, wo, bo, mask):
    """Fallback for masks the fast path does not handle (non-causal)."""
    out = np.empty((B, S, D), np.float32)
    scale = 1.0 / np.sqrt(DK)
    for b in range(B):
        Q = (q[b] @ wq.T + bq).reshape(S, H, DK).transpose(1, 0, 2)
        K = (k[b] @ wk.T + bk).reshape(S, H, DK).transpose(1, 0, 2)
        V = (v[b] @ wv.T + bv).reshape(S, H, DK).transpose(1, 0, 2)
        ctx = np.empty((H, S, DK), np.float32)
        for h in range(H):
            s = (Q[h] @ K[h].T) * scale
            s = np.where(mask, s, -1e9)
            s -= s.max(axis=-1, keepdims=True)
            e = np.exp(s)
            p = e / e.sum(axis=-1, keepdims=True)
            ctx[h] = p @ V[h]
        out[b] = ctx.transpose(1, 0, 2).reshape(S, D) @ wo.T + bo
    return out


def _prepare_in_maps(q, k, v, wq, bq, wk, bk, wv, bv, wo):
    # causal 0/1 diagonal-block masks: maskc[k, j*512+q] = (128j + k) <= q
    kk = np.arange(P)[:, None]
    qq = np.arange(512)[None, :]
    maskc = np.zeros((P, 4, 512), np.float32)
    for j in range(4):
        maskc[:, j, :] = (P * j + kk) <= qq
    maskc = np.ascontiguousarray(maskc.reshape(P, 4 * 512)).astype(BF)

    scale = 1.0 / np.sqrt(DK)
    wqT = np.ascontiguousarray(wq.T) * scale      # [d_in, e_out], pre-scaled
    wkT = np.ascontiguousarray(wk.T)
    wvT = np.ascontiguousarray(wv.T)
    woT = np.ascontiguousarray(wo.T)              # [e_in, d_out]

    qTb = [np.ascontiguousarray(q[b].T).astype(BF) for b in range(B)]
    kTb = [np.ascontiguousarray(k[b].T).astype(BF) for b in range(B)]
    vTb = [np.ascontiguousarray(v[b].T).astype(BF) for b in range(B)]

    in_maps = []
    for c in range(NCORES):
        b = c // 4
        e0 = 3 * (c % 4) * DK
        wq_c = wqT[:, e0:e0 + E]
        wk_c = wkT[:, e0:e0 + E]
        bq_c = bq[e0:e0 + E] * scale
        bk_c = bk[e0:e0 + E]
        wqk1 = np.concatenate([wq_c[:, P:E], wk_c[:, P:E]], axis=1)
        bqk1 = np.concatenate([bq_c[P:E], bk_c[P:E]])[:, None]
        wvp = np.ascontiguousarray(wvT[:, e0:e0 + E]).astype(BF)
        bvp = np.zeros((P, 192), np.float32)
        bvp[:E, :] = 0.0
        bvp = np.ascontiguousarray(
            np.broadcast_to(bv[e0:e0 + E][None, :], (P, 192))
        ).astype(np.float32)
        in_maps.append({
            "qT": qTb[b],
            "kT": kTb[b],
            "vT": vTb[b],
            "wq0": np.ascontiguousarray(wq_c[:, 0:P]).astype(BF),
            "wk0": np.ascontiguousarray(wk_c[:, 0:P]).astype(BF),
            "wqk1": np.ascontiguousarray(wqk1).astype(BF),
            "wvT": wvp,
            "woT": np.ascontiguousarray(woT[e0:e0 + E, :]).astype(BF),
            "bq0": np.ascontiguousarray(bq_c[0:P])[:, None].astype(np.float32),
            "bk0": np.ascontiguousarray(bk_c[0:P])[:, None].astype(np.float32),
            "bqk1": np.ascontiguousarray(bqk1).astype(np.float32),
            "bv": bvp,
            "maskc": maskc,
        })
    return in_maps


def kernel(q, k, v, wq, bq, wk, bk, wv, bv, wo, bo, mask, **_unused):
    q = np.asarray(q, np.float32)
    k = np.asarray(k, np.float32)
    v = np.asarray(v, np.float32)
    wq = np.asarray(wq, np.float32)
    wk = np.asarray(wk, np.float32)
    wv = np.asarray(wv, np.float32)
    wo = np.asarray(wo, np.float32)
    bq = np.asarray(bq, np.float32)
    bk = np.asarray(bk, np.float32)
    bv = np.asarray(bv, np.float32)
    bo = np.asarray(bo, np.float32)
    mask = np.asarray(mask)

    tril = np.tril(np.ones((S, S), bool))
    if mask.shape != (S, S) or not np.array_equal(mask.astype(bool), tril):
        return _numpy_reference(q, k, v, wq, bq, wk, bk, wv, bv, wo, bo, mask)

    in_maps = _prepare_in_maps(q, k, v, wq, bq, wk, bk, wv, bv, wo)
    nc = _get_nc()
    res = run_bass_kernel_spmd(nc, in_maps, core_ids=list(range(NCORES)))

    out = np.empty((B, S, D), np.float32)
    for b in range(B):
        acc = res.results[4 * b]["out_p"].astype(np.float32)
        for c in range(4 * b + 1, 4 * b + 4):
            acc = acc + res.results[c]["out_p"].astype(np.float32)
        out[b] = acc + bo[None, :]
    return out
